# revision 6
# baseline (speedup 1.0000x reference)
"""LowRankSparse2to4Linear Trainium2 kernel (v2).

out = (x16 @ A16) -> fp16 -> (@ B16^T) + bias, where A16/B16 are the 2:4
soft-thresholded (along rank), scaled, fp16-cast low-rank factors.

Strategy (8 NeuronCores, data-parallel over tokens, NO collectives):
  - tokens (8192) sharded 1024/core; every core receives the FULL weights
    and redundantly preprocesses them on-chip.
  - ZERO tensor-engine transposes: both x^T and B^T are produced by the
    DMA xbar transpose (InstDmaTransposeAnt, 16x128 tiles) straight into
    matmul-ready SBUF layouts. The PE runs a pure back-to-back stream of
    1024 N=512 fp16 matmuls (GEMM1 + GEMM2), staying HAM-warm.
  - 2:4 soft-threshold: ACT casts each f32 weight chunk-pair to fp16 once
    (no Abs passes -> no activation-table thrash); DVE then runs 6 fused
    ops per 2-chunk batch: MINABS/MAXABS customs (pairwise |.| min/max),
    max/min/min (2nd-smallest magnitude t), and a fused SOFT_SHRINK.
    The rank permutation introduced by the deinterleaved layout is
    identical for A and B^T so it cancels in GEMM2's contraction.
  - GEMM1 computes x_proj^T = A_sp^T @ x^T th-half by th-half (8 PSUM
    banks, ic-paced to match DVE sparsify throughput); GEMM2 consumes
    x_proj^T as the stationary operand against xbar-produced B^T tiles.
  - SBUF: the two 32KB B^T tiles reuse the two 32KB x^T tiles' buffers
    (freed when each GEMM1 th-half retires).
  - Output is written fp16 (halves store traffic); host upcasts.
"""

import os
import sys
import numpy as np

sys.path.insert(0, "/opt/trn_rl_repo")

N_CORES = 8
IN_F, OUT_F, RANK = 4096, 4096, 1024
T_FULL = 8192             # 4 * 2048 tokens
TPC = T_FULL // N_CORES   # 1024 tokens per core

_BUILD_CACHE = {}


_DVE_OPS = {}


def _register_custom_dve_ops():
    """Register fused DVE ops (runtime extension of concourse.dve_ops).

    MINABS/MAXABS: out = min/max(|in0|, |in1|)
    SOFT_SHRINK:   out = in0 - clamp(in0, -in1, in1)   (in1 >= 0)
    """
    if _DVE_OPS:
        return _DVE_OPS
    import numpy as _np
    from concourse import dve_ops
    from concourse.dve_spec import (Spec, Src0, Src1, Zero, minn, maxx,
                                    select, lower, _has_src1)
    from concourse.dve_uop import DveOpSpec

    def make_op(name, body, ref):
        existing = {op.name: op for op in dve_ops.OPS}
        if name in existing:
            return existing[name]
        spec = Spec(body=body, reference=ref)
        row = dve_ops._CUSTOM_DVE_ROW_BASE + len(dve_ops.OPS)
        shas = {}
        for ver in ("v3", "v4"):
            try:
                tmp = DveOpSpec(name=name, opcode=row, uops=lower(spec, ver=ver),
                                rd1_en=_has_src1(spec))
                shas[ver] = tmp.sha(ver)
            except Exception:
                pass
        op = dve_ops.DveOp(name, spec, subdim=False, uops_sha=shas)
        dve_ops.OPS.append(op)
        dve_ops.CUSTOM_DVE_SPECS[name] = spec
        dve_ops._SUB_OPCODE_FOR_NAME[name] = row
        return op

    _DVE_OPS["minabs"] = make_op(
        "MINABS_ANT", minn(maxx(Src0, Zero - Src0), maxx(Src1, Zero - Src1)),
        lambda in0, in1, s0, s1, imm2: _np.minimum(_np.abs(in0), _np.abs(in1)))
    _DVE_OPS["maxabs"] = make_op(
        "MAXABS_ANT", maxx(maxx(Src0, Zero - Src0), maxx(Src1, Zero - Src1)),
        lambda in0, in1, s0, s1, imm2: _np.maximum(_np.abs(in0), _np.abs(in1)))
    _DVE_OPS["shrink"] = make_op(
        "SOFT_SHRINK_ANT",
        select(Src0 < Zero, minn(Src0 + Src1, Zero), maxx(Src0 - Src1, Zero)),
        lambda in0, in1, s0, s1, imm2: _np.where(
            in0 < 0, _np.minimum(in0 + in1, 0), _np.maximum(in0 - in1, 0)))
    return _DVE_OPS


def _build(scale_a: float, scale_b: float, bias_zero: bool):
    import concourse.bacc as bacc
    import concourse.tile as tile
    from concourse import mybir

    ops = _register_custom_dve_ops()

    f32 = mybir.dt.float32
    f16 = mybir.dt.float16
    Alu = mybir.AluOpType

    nc = bacc.Bacc("TRN2", target_bir_lowering=False, debug=False,
                   num_devices=N_CORES)

    x_sh = nc.dram_tensor("x_sh", [TPC, IN_F], f32, kind="ExternalInput")
    wa_d = nc.dram_tensor("wa_d", [IN_F, RANK], f32, kind="ExternalInput")
    wb_d = nc.dram_tensor("wb_d", [OUT_F, RANK], f32, kind="ExternalInput")
    bias_d = nc.dram_tensor("bias_d", [1, OUT_F], f32, kind="ExternalInput")
    out_d = nc.dram_tensor("out_d", [TPC, OUT_F], f16, kind="ExternalOutput")

    K_IN = IN_F // 128    # 32 contraction chunks for GEMM1
    K_RK = RANK // 128    # 8 contraction chunks for GEMM2
    NB_W = 16             # weight sparsify batches per matrix (2 chunks each)

    with tile.TileContext(nc) as tc:
        with (
            tc.tile_pool(name="wst", bufs=2) as p_wst,       # 2x8KB f32 stage
            tc.tile_pool(name="g16", bufs=2) as p_g16,       # 2x4KB f16 cast
            tc.tile_pool(name="pq", bufs=2) as p_pq,         # P,Q 2x2KB
            tc.tile_pool(name="eft", bufs=4) as p_eft,       # E,F,t 4x1KB
            tc.tile_pool(name="wasp", bufs=NB_W) as p_wasp,  # A_sp 16x4KB
            tc.tile_pool(name="wbsp", bufs=2) as p_wbsp,     # B_sp stage 2x4KB
            tc.tile_pool(name="xf", bufs=2) as p_xf,         # x f32 2x4KB
            tc.tile_pool(name="x16", bufs=2) as p_x16,       # x f16 2x2KB
            tc.tile_pool(name="big", bufs=2) as p_big,       # XT0/XT1 -> WBTa/b
            tc.tile_pool(name="xproj", bufs=16) as p_xp,     # 16x1KB
            tc.tile_pool(name="o16", bufs=4) as p_o16,       # out stage 4x1KB
            tc.tile_pool(name="ps", bufs=8, space="PSUM") as p_ps,
        ):
            # ---------------- weight sparsify (2 chunks per batch) --------
            def sparsify_dma(src_dram, b, tag):
                """Issue the two chunk DMAs for batch b (rows [256b,+256))."""
                wst = p_wst.tile([128, 2048], f32, tag="wst", name=f"wst_{tag}{b}")
                nc.scalar.dma_start(wst[:, 0:1024],
                                    src_dram[256 * b:256 * b + 128, :])
                nc.scalar.dma_start(wst[:, 1024:2048],
                                    src_dram[256 * b + 128:256 * b + 256, :])
                return wst

            def sparsify_compute(wst, b, scale, dst_pool, tag):
                """2:4 soft-threshold one staged (128, 2048) f32 batch into a
                (128, 2048) fp16 tile: [:, c*1024 + l*256 + q] holds
                soft(w)[256b+c*128+p, 4q+l].  Permuted-rank (c,l,q) layout;
                the same permutation is used for A and B so it cancels in
                GEMM2's contraction."""
                if scale != 1.0:
                    nc.scalar.mul(wst[:], wst[:], float(scale))
                # ACT: deinterleave + cast, one rank-3 pass per lane l.
                g16 = p_g16.tile([128, 2048], f16, tag="g16", name=f"g16_{tag}{b}")
                wv = wst[:].rearrange("p (c q l) -> p c l q", c=2, q=256, l=4)
                gv = g16[:].rearrange("p (c l q) -> p c l q", c=2, l=4, q=256)
                for l in range(4):
                    nc.scalar.copy(gv[:, :, l, :], wv[:, :, l, :])
                # DVE: pairwise |.| min/max on contiguous lane blocks.
                gf = g16[:].rearrange("p (c h) -> p c h", c=2, h=1024)
                P = p_pq.tile([128, 1024], f16, tag="pq", name=f"P_{tag}{b}")
                Q = p_pq.tile([128, 1024], f16, tag="pq", name=f"Q_{tag}{b}")
                pv = P[:].rearrange("p (c s) -> p c s", c=2, s=512)
                qv = Q[:].rearrange("p (c s) -> p c s", c=2, s=512)
                nc.vector._custom_dve(ops["minabs"], out=pv,
                                      in0=gf[:, :, 0:512], in1=gf[:, :, 512:1024])
                nc.vector._custom_dve(ops["maxabs"], out=qv,
                                      in0=gf[:, :, 0:512], in1=gf[:, :, 512:1024])
                # t = 2nd-smallest magnitude per group of 4:
                #   E = max of pair-mins, F = min of pair-maxes, t = min(E,F)
                E = p_eft.tile([128, 512], f16, tag="eft", name=f"E_{tag}{b}")
                F = p_eft.tile([128, 512], f16, tag="eft", name=f"F_{tag}{b}")
                t = p_eft.tile([128, 512], f16, tag="eft", name=f"t_{tag}{b}")
                pj = P[:].rearrange("p (c j q) -> p c j q", c=2, j=2, q=256)
                qj = Q[:].rearrange("p (c j q) -> p c j q", c=2, j=2, q=256)
                ev = E[:].rearrange("p (c q) -> p c q", c=2)
                fv = F[:].rearrange("p (c q) -> p c q", c=2)
                nc.vector.tensor_tensor(out=ev, in0=pj[:, :, 0, :],
                                        in1=pj[:, :, 1, :], op=Alu.max)
                nc.vector.tensor_tensor(out=fv, in0=qj[:, :, 0, :],
                                        in1=qj[:, :, 1, :], op=Alu.min)
                nc.vector.tensor_tensor(out=t[:], in0=E[:], in1=F[:], op=Alu.min)
                wsp = dst_pool.tile([128, 2048], f16, tag=tag,
                                    name=f"wsp_{tag}{b}")
                for c in range(2):
                    nc.vector._custom_dve(
                        ops["shrink"],
                        out=wsp[:, c * 1024:(c + 1) * 1024].rearrange(
                            "p (l q) -> p l q", l=4, q=256),
                        in0=gf[:, c, :].rearrange("p (l q) -> p l q", l=4, q=256),
                        in1=t[:, c * 256:(c + 1) * 256][:, None, :]
                        .to_broadcast([128, 4, 256]))
                return wsp

            # ---------------- x load / cast / xbar-transpose --------------
            # XT[th] viewed (128, tb=4, ic=32, t=128):
            #   XT[th][i, tb, ic, t] = x16[th*512 + tb*128 + t, ic*128 + i]
            XT = [p_big.tile([128, 16384], f16, tag="big", name=f"XT{th}")
                  for th in range(2)]

            def emit_x_unit(th, tb, q):
                """Load+cast+transpose x rows [512th+128tb, +128), in-cols
                [1024q, +1024) into XT[th][:, tb, 8q:8q+8, :]."""
                tok0 = th * 512 + tb * 128
                xf = p_xf.tile([128, 1024], f32, tag="xf",
                               name=f"xf_{th}_{tb}_{q}")
                nc.sync.dma_start(xf[:], x_sh[tok0:tok0 + 128,
                                              1024 * q:1024 * (q + 1)])
                x16t = p_x16.tile([128, 1024], f16, tag="x16",
                                  name=f"x16_{th}_{tb}_{q}")
                nc.scalar.copy(x16t[:], xf[:])
                xtv = XT[th][:].rearrange("p (tb ic t) -> p tb ic t",
                                          tb=4, ic=32, t=128)
                nc.sync.dma_start_transpose(xtv[:, tb, 8 * q:8 * (q + 1), :],
                                            x16t[:])

            # Emission: interleave A sparsify batches with x units so the DMA
            # queues and ACT run both pipelines concurrently.  x units are
            # q-major so GEMM1 can start after only the first 4 units of th0
            # (they cover in-chunks 0..7 for all tb).  Weight DMAs are issued
            # one batch ahead so transfers hide under the previous batch's
            # ACT/DVE work instead of blocking the ACT queue.
            wa_sp = []
            x_units = [(th, tb, q) for th in range(2) for q in range(4)
                       for tb in range(4)]
            xi = 0
            wst_pend = sparsify_dma(wa_d, 0, "wa")
            for b in range(NB_W):
                while xi < (b + 1) * 2 and xi < len(x_units):
                    emit_x_unit(*x_units[xi])
                    xi += 1
                wst_cur = wst_pend
                if b + 1 < NB_W:
                    wst_pend = sparsify_dma(wa_d, b + 1, "wa")
                wa_sp.append(
                    sparsify_compute(wst_cur, b, scale_a, p_wasp, "wa"))
            while xi < len(x_units):
                emit_x_unit(*x_units[xi])
                xi += 1

            # ---------------- bias broadcast (only if nonzero) ------------
            if not bias_zero:
                bias_pool = tc.tile_pool(name="bias", bufs=1)
                bias_pool.__enter__()
                bias_bc = bias_pool.tile([128, OUT_F], f32)
                nc.sync.dma_start(bias_bc[0:1, :], bias_d[:])
                k = 1
                while k < 128:
                    nc.sync.dma_start(bias_bc[k:2 * k, :], bias_bc[0:k, :])
                    k *= 2

            # ---------------- GEMM1: x_proj^T = A_sp^T @ x^T --------------
            # th-outer so th0 only needs XT0 and chunk consumption (55us)
            # matches DVE sparsify pace; 8 PSUM banks per th-half.
            xproj = {}  # (th, kc) -> (128 rank', 512 tok) fp16

            def gemm1_half(th):
                xtv = XT[th][:].rearrange("p (tb ic t) -> p tb ic t",
                                          tb=4, ic=32, t=128)
                accs = [p_ps.tile([128, 512], f32, tag="ps",
                                  name=f"g1_{th}_{m}") for m in range(8)]
                for ic in range(K_IN):
                    wsp = wa_sp[ic // 2]
                    c0 = (ic % 2) * 1024
                    for m in range(8):
                        nc.tensor.matmul(
                            accs[m][:],
                            wsp[:, c0 + m * 128:c0 + (m + 1) * 128],
                            xtv[:, :, ic, :],
                            start=(ic == 0), stop=(ic == K_IN - 1))
                return accs

            def xproj_copies(th, accs):
                for m in range(8):
                    xp = p_xp.tile([128, 512], f16, tag="xp",
                                   name=f"xp_{th}_{m}")
                    nc.scalar.copy(xp[:], accs[m][:])
                    xproj[(th, m)] = xp

            accs0 = gemm1_half(0)
            xproj_copies(0, accs0)

            # ---------------- B sparsify + xbar into WBT ------------------
            # WBT_x viewed (128, ob=16, rc=8, o=128):
            #   WBT_x[r, ob, rc, o] = B_sp^T[rc*128 + r, ob*128 + o]
            # WBT_a (out cols 0..2047) reuses XT0's buffer, WBT_b XT1's.
            WBT = [None, None]
            wbt_views = [None, None]

            def emit_b_batches(lo, hi, wbt_idx):
                wst_p = sparsify_dma(wb_d, lo, "wb")
                for b in range(lo, hi):
                    wst_c = wst_p
                    if b + 1 < hi:
                        wst_p = sparsify_dma(wb_d, b + 1, "wb")
                    wsp = sparsify_compute(wst_c, b, scale_b, p_wbsp, "wbsp")
                    wv = wsp[:].rearrange("p (c r) -> p c r", c=2, r=1024)
                    for c in range(2):
                        ob = 2 * b + c - wbt_idx * 16
                        nc.sync.dma_start_transpose(
                            wbt_views[wbt_idx][:, ob, :, :], wv[:, c, :])

            WBT[0] = p_big.tile([128, 16384], f16, tag="big", name="WBTa")
            wbt_views[0] = WBT[0][:].rearrange("p (ob rc o) -> p ob rc o",
                                               ob=16, rc=8, o=128)
            emit_b_batches(0, 8, 0)

            accs1 = gemm1_half(1)

            WBT[1] = p_big.tile([128, 16384], f16, tag="big", name="WBTb")
            wbt_views[1] = WBT[1][:].rearrange("p (ob rc o) -> p ob rc o",
                                               ob=16, rc=8, o=128)
            emit_b_batches(8, 16, 1)

            xproj_copies(1, accs1)

            # ---------------- GEMM2: out = x_proj @ B_sp^T + bias ---------
            for nb in range(OUT_F // 512):
                wv = wbt_views[nb // 4]
                nbl = nb % 4
                for mt in range(TPC // 128):
                    th, ml = mt // 4, mt % 4
                    acc2 = p_ps.tile([128, 512], f32, tag="ps",
                                     name=f"g2_{nb}_{mt}")
                    for kc in range(K_RK):
                        nc.tensor.matmul(
                            acc2[:],
                            xproj[(th, kc)][:, ml * 128:(ml + 1) * 128],
                            wv[:, 4 * nbl:4 * (nbl + 1), kc, :],
                            start=(kc == 0), stop=(kc == K_RK - 1))
                    ot = p_o16.tile([128, 512], f16, tag="o16",
                                    name=f"ot_{nb}_{mt}")
                    if bias_zero:
                        nc.vector.tensor_copy(out=ot[:], in_=acc2[:])
                    else:
                        nc.vector.tensor_tensor(
                            out=ot[:], in0=acc2[:],
                            in1=bias_bc[:, nb * 512:(nb + 1) * 512],
                            op=Alu.add)
                    nc.sync.dma_start(
                        out_d[mt * 128:(mt + 1) * 128,
                              nb * 512:(nb + 1) * 512],
                        ot[:])

    nc.compile()
    return nc


def kernel(x, weight_A, weight_B, bias, scale_A, scale_B):
    from concourse.bass_utils import run_bass_kernel_spmd

    x = np.ascontiguousarray(np.asarray(x, dtype=np.float32))
    weight_A = np.ascontiguousarray(np.asarray(weight_A, dtype=np.float32))
    weight_B = np.ascontiguousarray(np.asarray(weight_B, dtype=np.float32))
    bias = np.ascontiguousarray(np.asarray(bias, dtype=np.float32))
    sa = float(np.asarray(scale_A))
    sb = float(np.asarray(scale_B))
    bias_zero = bool(np.all(bias == 0.0))

    lead = x.shape[:-1]
    xf = x.reshape(-1, IN_F)
    assert xf.shape == (T_FULL, IN_F)

    key = (sa, sb, bias_zero)
    if key not in _BUILD_CACHE:
        _BUILD_CACHE[key] = _build(sa, sb, bias_zero)
    nc = _BUILD_CACHE[key]

    bias_row = bias.reshape(1, OUT_F)
    in_maps = []
    for c in range(N_CORES):
        in_maps.append({
            "x_sh": xf[c * TPC:(c + 1) * TPC],
            "wa_d": weight_A,
            "wb_d": weight_B,
            "bias_d": bias_row,
        })

    trace = os.environ.get("BASS_KERNEL_TRACE", "0") == "1"
    kwargs = {}
    if trace:
        _install_ntff_hook()
        kwargs["trace"] = True
        tmpdir = os.environ.get("BASS_KERNEL_TRACE_DIR")
        if tmpdir:
            os.makedirs(tmpdir, exist_ok=True)
            kwargs["tmpdir"] = tmpdir

    res = run_bass_kernel_spmd(nc, in_maps, core_ids=list(range(N_CORES)),
                               **kwargs)
    if trace:
        kernel.last_exec_time_ns = res.exec_time_ns

    out = np.empty((T_FULL, OUT_F), dtype=np.float32)
    for c in range(N_CORES):
        out[c * TPC:(c + 1) * TPC] = res.results[c]["out_d"]
    return out.reshape(*lead, OUT_F)


def _install_ntff_hook():
    """Provide antenv.axon_hooks (missing in this image) so trace=True works."""
    import types
    if "antenv.axon_hooks" in sys.modules:
        return
    try:
        from trn_agent_boot.trn_boot import _ntff_profile_via_ctypes
        hook = _ntff_profile_via_ctypes("/opt/axon/libaxon_pjrt.so")
    except Exception:
        hook = None
    mod = types.ModuleType("antenv.axon_hooks")
    mod.get_axon_ntff_profile_hook = lambda: hook
    mod.set_axon_ntff_profile_hook = lambda h: None
    import antenv  # noqa: F401
    sys.modules["antenv.axon_hooks"] = mod


# revision 13
# speedup vs baseline: 1.0655x; 1.0655x over previous
"""LowRankSparse2to4Linear Trainium2 kernel (v2).

out = (x16 @ A16) -> fp16 -> (@ B16^T) + bias, where A16/B16 are the 2:4
soft-thresholded (along rank), scaled, fp16-cast low-rank factors.

Strategy (8 NeuronCores, data-parallel over tokens, NO collectives):
  - tokens (8192) sharded 1024/core; every core receives the FULL weights
    and redundantly preprocesses them on-chip.
  - ZERO tensor-engine transposes: both x^T and B^T are produced by the
    DMA xbar transpose (InstDmaTransposeAnt, 16x128 tiles) straight into
    matmul-ready SBUF layouts. The PE runs a pure back-to-back stream of
    1024 N=512 fp16 matmuls (GEMM1 + GEMM2), staying HAM-warm.
  - 2:4 soft-threshold: ACT casts each f32 weight chunk-pair to fp16 once
    (no Abs passes -> no activation-table thrash); DVE then runs 6 fused
    ops per 2-chunk batch: MINABS/MAXABS customs (pairwise |.| min/max),
    max/min/min (2nd-smallest magnitude t), and a fused SOFT_SHRINK.
    The rank permutation introduced by the deinterleaved layout is
    identical for A and B^T so it cancels in GEMM2's contraction.
  - GEMM1 computes x_proj^T = A_sp^T @ x^T th-half by th-half (8 PSUM
    banks, ic-paced to match DVE sparsify throughput); GEMM2 consumes
    x_proj^T as the stationary operand against xbar-produced B^T tiles.
  - SBUF: the two 32KB B^T tiles reuse the two 32KB x^T tiles' buffers
    (freed when each GEMM1 th-half retires).
  - Output is written fp16 (halves store traffic); host upcasts.
"""

import os
import sys
import numpy as np

sys.path.insert(0, "/opt/trn_rl_repo")

N_CORES = 8
IN_F, OUT_F, RANK = 4096, 4096, 1024
T_FULL = 8192             # 4 * 2048 tokens
TPC = T_FULL // N_CORES   # 1024 tokens per core

_BUILD_CACHE = {}


_DVE_OPS = {}


def _register_custom_dve_ops():
    """Register fused DVE ops (runtime extension of concourse.dve_ops).

    MINABS/MAXABS: out = min/max(|in0|, |in1|)
    SOFT_SHRINK:   out = in0 - clamp(in0, -in1, in1)   (in1 >= 0)
    """
    if _DVE_OPS:
        return _DVE_OPS
    import numpy as _np
    from concourse import dve_ops
    from concourse.dve_spec import (Spec, Src0, Src1, Zero, minn, maxx,
                                    select, lower, _has_src1)
    from concourse.dve_uop import DveOpSpec

    def make_op(name, body, ref):
        existing = {op.name: op for op in dve_ops.OPS}
        if name in existing:
            return existing[name]
        spec = Spec(body=body, reference=ref)
        row = dve_ops._CUSTOM_DVE_ROW_BASE + len(dve_ops.OPS)
        shas = {}
        for ver in ("v3", "v4"):
            try:
                tmp = DveOpSpec(name=name, opcode=row, uops=lower(spec, ver=ver),
                                rd1_en=_has_src1(spec))
                shas[ver] = tmp.sha(ver)
            except Exception:
                pass
        op = dve_ops.DveOp(name, spec, subdim=False, uops_sha=shas)
        dve_ops.OPS.append(op)
        dve_ops.CUSTOM_DVE_SPECS[name] = spec
        dve_ops._SUB_OPCODE_FOR_NAME[name] = row
        return op

    _DVE_OPS["minabs"] = make_op(
        "MINABS_ANT", minn(maxx(Src0, Zero - Src0), maxx(Src1, Zero - Src1)),
        lambda in0, in1, s0, s1, imm2: _np.minimum(_np.abs(in0), _np.abs(in1)))
    _DVE_OPS["maxabs"] = make_op(
        "MAXABS_ANT", maxx(maxx(Src0, Zero - Src0), maxx(Src1, Zero - Src1)),
        lambda in0, in1, s0, s1, imm2: _np.maximum(_np.abs(in0), _np.abs(in1)))
    _DVE_OPS["shrink"] = make_op(
        "SOFT_SHRINK_ANT",
        select(Src0 < Zero, minn(Src0 + Src1, Zero), maxx(Src0 - Src1, Zero)),
        lambda in0, in1, s0, s1, imm2: _np.where(
            in0 < 0, _np.minimum(in0 + in1, 0), _np.maximum(in0 - in1, 0)))
    return _DVE_OPS


def _build(scale_a: float, scale_b: float, bias_zero: bool):
    import concourse.bacc as bacc
    import concourse.tile as tile
    from concourse import mybir

    ops = _register_custom_dve_ops()

    f32 = mybir.dt.float32
    f16 = mybir.dt.float16
    Alu = mybir.AluOpType

    nc = bacc.Bacc("TRN2", target_bir_lowering=False, debug=False,
                   num_devices=N_CORES)

    x_sh = nc.dram_tensor("x_sh", [TPC, IN_F], f32, kind="ExternalInput")
    wa_d = nc.dram_tensor("wa_d", [IN_F, RANK], f32, kind="ExternalInput")
    wb_d = nc.dram_tensor("wb_d", [OUT_F, RANK], f32, kind="ExternalInput")
    bias_d = nc.dram_tensor("bias_d", [1, OUT_F], f32, kind="ExternalInput")
    out_d = nc.dram_tensor("out_d", [TPC, OUT_F], f16, kind="ExternalOutput")

    K_IN = IN_F // 128    # 32 contraction chunks for GEMM1
    K_RK = RANK // 128    # 8 contraction chunks for GEMM2
    NB_W = 16             # weight sparsify batches per matrix (2 chunks each)

    with tile.TileContext(nc) as tc:
        with (
            tc.tile_pool(name="wst", bufs=2) as p_wst,       # 2x8KB f32 stage
            tc.tile_pool(name="g16", bufs=2) as p_g16,       # 2x4KB f16 cast
            tc.tile_pool(name="pq", bufs=2) as p_pq,         # P,Q 2x2KB
            tc.tile_pool(name="eft", bufs=3) as p_eft,       # E,F,t 4x1KB
            tc.tile_pool(name="wasp", bufs=NB_W) as p_wasp,  # A_sp 16x4KB
            tc.tile_pool(name="wbsp", bufs=2) as p_wbsp,     # B_sp stage 2x4KB
            tc.tile_pool(name="xf", bufs=2) as p_xf,         # x f32 2x4KB
            tc.tile_pool(name="x16", bufs=2) as p_x16,       # x f16 2x2KB
            tc.tile_pool(name="big", bufs=2) as p_big,       # XT0/XT1 -> WBTa/b
            tc.tile_pool(name="xproj", bufs=16) as p_xp,     # 16x1KB
            tc.tile_pool(name="o16", bufs=3) as p_o16,       # out stage 4x1KB
            tc.tile_pool(name="ps", bufs=8, space="PSUM") as p_ps,
        ):
            # ---------------- weight sparsify (2 chunks per batch) --------
            def sparsify_dma(src_dram, b, tag):
                """Issue the two chunk DMAs for batch b (rows [256b,+256))."""
                wst = p_wst.tile([128, 2048], f32, tag="wst", name=f"wst_{tag}{b}")
                nc.scalar.dma_start(wst[:, 0:1024],
                                    src_dram[256 * b:256 * b + 128, :])
                nc.scalar.dma_start(wst[:, 1024:2048],
                                    src_dram[256 * b + 128:256 * b + 256, :])
                return wst

            def sparsify_compute(wst, b, scale, dst_pool, tag):
                """2:4 soft-threshold one staged (128, 2048) f32 batch into a
                (128, 2048) fp16 tile in NATURAL rank layout (no permutation):
                wsp[p, r] = soft(w)[256b + c*128 + p, r mod 1024].
                One contiguous ACT cast; stride-2 pair reductions on DVE with
                adjacent pairing (0,1),(2,3); E/F on the otherwise-idle
                GPSIMD; single fused SOFT_SHRINK for the whole batch."""
                if scale != 1.0:
                    nc.scalar.mul(wst[:], wst[:], float(scale))
                g16 = p_g16.tile([128, 2048], f16, tag="g16", name=f"g16_{tag}{b}")
                nc.scalar.copy(g16[:], wst[:])
                gp = g16[:].rearrange("p (g two) -> p g two", g=1024, two=2)
                P = p_pq.tile([128, 1024], f16, tag="pq", name=f"P_{tag}{b}")
                Q = p_pq.tile([128, 1024], f16, tag="pq", name=f"Q_{tag}{b}")
                nc.vector._custom_dve(ops["minabs"], out=P[:],
                                      in0=gp[:, :, 0], in1=gp[:, :, 1])
                nc.vector._custom_dve(ops["maxabs"], out=Q[:],
                                      in0=gp[:, :, 0], in1=gp[:, :, 1])
                # t = 2nd-smallest magnitude per group of 4:
                #   E = max of pair-mins, F = min of pair-maxes, t = min(E,F)
                E = p_eft.tile([128, 512], f16, tag="eft", name=f"E_{tag}{b}")
                F = p_eft.tile([128, 512], f16, tag="eft", name=f"F_{tag}{b}")
                t = p_eft.tile([128, 512], f16, tag="eft", name=f"t_{tag}{b}")
                pj = P[:].rearrange("p (g j) -> p g j", g=512, j=2)
                qj = Q[:].rearrange("p (g j) -> p g j", g=512, j=2)
                nc.vector.tensor_tensor(out=E[:], in0=pj[:, :, 0],
                                        in1=pj[:, :, 1], op=Alu.max)
                nc.vector.tensor_tensor(out=F[:], in0=qj[:, :, 0],
                                        in1=qj[:, :, 1], op=Alu.min)
                nc.vector.tensor_tensor(out=t[:], in0=E[:], in1=F[:], op=Alu.min)
                wsp = dst_pool.tile([128, 2048], f16, tag=tag,
                                    name=f"wsp_{tag}{b}")
                nc.vector._custom_dve(
                    ops["shrink"],
                    out=wsp[:].rearrange("p (g l) -> p g l", g=512, l=4),
                    in0=g16[:].rearrange("p (g l) -> p g l", g=512, l=4),
                    in1=t[:, :, None].to_broadcast([128, 512, 4]))
                return wsp

            # ---------------- x load / cast / xbar-transpose --------------
            # XT[th] viewed (128, tb=4, ic=32, t=128):
            #   XT[th][i, tb, ic, t] = x16[th*512 + tb*128 + t, ic*128 + i]
            XT = [p_big.tile([128, 16384], f16, tag="big", name=f"XT{th}")
                  for th in range(2)]

            def emit_x_unit(th, tb, q):
                """Load+cast+transpose x rows [512th+128tb, +128), in-cols
                [1024q, +1024) into XT[th][:, tb, 8q:8q+8, :]."""
                tok0 = th * 512 + tb * 128
                xf = p_xf.tile([128, 1024], f32, tag="xf",
                               name=f"xf_{th}_{tb}_{q}")
                nc.sync.dma_start(xf[:], x_sh[tok0:tok0 + 128,
                                              1024 * q:1024 * (q + 1)])
                x16t = p_x16.tile([128, 1024], f16, tag="x16",
                                  name=f"x16_{th}_{tb}_{q}")
                nc.scalar.copy(x16t[:], xf[:])
                xtv = XT[th][:].rearrange("p (tb ic t) -> p tb ic t",
                                          tb=4, ic=32, t=128)
                nc.sync.dma_start_transpose(xtv[:, tb, 8 * q:8 * (q + 1), :],
                                            x16t[:])

            # Emission: interleave A sparsify batches with x units so the DMA
            # queues and ACT run both pipelines concurrently.  x units are
            # q-major so GEMM1 can start after only the first 4 units of th0
            # (they cover in-chunks 0..7 for all tb).  Weight DMAs are issued
            # one batch ahead so transfers hide under the previous batch's
            # ACT/DVE work instead of blocking the ACT queue.
            wa_sp = []
            x_units = [(th, tb, q) for th in range(2) for q in range(4)
                       for tb in range(4)]
            xi = 0
            wst_pend = sparsify_dma(wa_d, 0, "wa")
            for b in range(NB_W):
                while xi < (b + 1) * 2 and xi < len(x_units):
                    emit_x_unit(*x_units[xi])
                    xi += 1
                wst_cur = wst_pend
                if b + 1 < NB_W:
                    wst_pend = sparsify_dma(wa_d, b + 1, "wa")
                wa_sp.append(
                    sparsify_compute(wst_cur, b, scale_a, p_wasp, "wa"))
            while xi < len(x_units):
                emit_x_unit(*x_units[xi])
                xi += 1

            # ---------------- bias broadcast (only if nonzero) ------------
            if not bias_zero:
                bias_pool = tc.tile_pool(name="bias", bufs=1)
                bias_pool.__enter__()
                bias_bc = bias_pool.tile([128, OUT_F], f32)
                nc.sync.dma_start(bias_bc[0:1, :], bias_d[:])
                k = 1
                while k < 128:
                    nc.sync.dma_start(bias_bc[k:2 * k, :], bias_bc[0:k, :])
                    k *= 2

            # ---------------- GEMM1: x_proj^T = A_sp^T @ x^T --------------
            # th-outer so th0 only needs XT0 and chunk consumption (55us)
            # matches DVE sparsify pace; 8 PSUM banks per th-half.
            xproj = {}  # (th, kc) -> (128 rank', 512 tok) fp16

            def gemm1_half(th):
                xtv = XT[th][:].rearrange("p (tb ic t) -> p tb ic t",
                                          tb=4, ic=32, t=128)
                accs = [p_ps.tile([128, 512], f32, tag="ps",
                                  name=f"g1_{th}_{m}") for m in range(8)]
                for ic in range(K_IN):
                    wsp = wa_sp[ic // 2]
                    c0 = (ic % 2) * 1024
                    for m in range(8):
                        nc.tensor.matmul(
                            accs[m][:],
                            wsp[:, c0 + m * 128:c0 + (m + 1) * 128],
                            xtv[:, :, ic, :],
                            start=(ic == 0), stop=(ic == K_IN - 1))
                return accs

            def xproj_copies(th, accs):
                for m in range(8):
                    xp = p_xp.tile([128, 512], f16, tag="xp",
                                   name=f"xp_{th}_{m}")
                    nc.scalar.copy(xp[:], accs[m][:])
                    xproj[(th, m)] = xp

            accs0 = gemm1_half(0)
            xproj_copies(0, accs0)

            # ---------------- B sparsify + xbar into WBT ------------------
            # WBT_x viewed (128, ob=16, rc=8, o=128):
            #   WBT_x[r, ob, rc, o] = B_sp^T[rc*128 + r, ob*128 + o]
            # WBT_a (out cols 0..2047) reuses XT0's buffer, WBT_b XT1's.
            WBT = [None, None]
            wbt_views = [None, None]

            def emit_b_batches(lo, hi, wbt_idx):
                wst_p = sparsify_dma(wb_d, lo, "wb")
                for b in range(lo, hi):
                    wst_c = wst_p
                    if b + 1 < hi:
                        wst_p = sparsify_dma(wb_d, b + 1, "wb")
                    wsp = sparsify_compute(wst_c, b, scale_b, p_wbsp, "wbsp")
                    wv = wsp[:].rearrange("p (c r) -> p c r", c=2, r=1024)
                    for c in range(2):
                        ob = 2 * b + c - wbt_idx * 16
                        nc.sync.dma_start_transpose(
                            wbt_views[wbt_idx][:, ob, :, :], wv[:, c, :])

            WBT[0] = p_big.tile([128, 16384], f16, tag="big", name="WBTa")
            wbt_views[0] = WBT[0][:].rearrange("p (ob rc o) -> p ob rc o",
                                               ob=16, rc=8, o=128)
            emit_b_batches(0, 8, 0)

            accs1 = gemm1_half(1)

            WBT[1] = p_big.tile([128, 16384], f16, tag="big", name="WBTb")
            wbt_views[1] = WBT[1][:].rearrange("p (ob rc o) -> p ob rc o",
                                               ob=16, rc=8, o=128)
            emit_b_batches(8, 16, 1)

            xproj_copies(1, accs1)

            # ---------------- GEMM2: out = x_proj @ B_sp^T + bias ---------
            for nb in range(OUT_F // 512):
                wv = wbt_views[nb // 4]
                nbl = nb % 4
                for mt in range(TPC // 128):
                    th, ml = mt // 4, mt % 4
                    acc2 = p_ps.tile([128, 512], f32, tag="ps",
                                     name=f"g2_{nb}_{mt}")
                    for kc in range(K_RK):
                        nc.tensor.matmul(
                            acc2[:],
                            xproj[(th, kc)][:, ml * 128:(ml + 1) * 128],
                            wv[:, 4 * nbl:4 * (nbl + 1), kc, :],
                            start=(kc == 0), stop=(kc == K_RK - 1))
                    ot = p_o16.tile([128, 512], f16, tag="o16",
                                    name=f"ot_{nb}_{mt}")
                    if bias_zero:
                        nc.scalar.copy(ot[:], acc2[:])
                    else:
                        nc.vector.tensor_tensor(
                            out=ot[:], in0=acc2[:],
                            in1=bias_bc[:, nb * 512:(nb + 1) * 512],
                            op=Alu.add)
                    nc.sync.dma_start(
                        out_d[mt * 128:(mt + 1) * 128,
                              nb * 512:(nb + 1) * 512],
                        ot[:])

    nc.compile()
    return nc


def kernel(x, weight_A, weight_B, bias, scale_A, scale_B):
    from concourse.bass_utils import run_bass_kernel_spmd

    x = np.ascontiguousarray(np.asarray(x, dtype=np.float32))
    weight_A = np.ascontiguousarray(np.asarray(weight_A, dtype=np.float32))
    weight_B = np.ascontiguousarray(np.asarray(weight_B, dtype=np.float32))
    bias = np.ascontiguousarray(np.asarray(bias, dtype=np.float32))
    sa = float(np.asarray(scale_A))
    sb = float(np.asarray(scale_B))
    bias_zero = bool(np.all(bias == 0.0))

    lead = x.shape[:-1]
    xf = x.reshape(-1, IN_F)
    assert xf.shape == (T_FULL, IN_F)

    key = (sa, sb, bias_zero)
    if key not in _BUILD_CACHE:
        _BUILD_CACHE[key] = _build(sa, sb, bias_zero)
    nc = _BUILD_CACHE[key]

    bias_row = bias.reshape(1, OUT_F)
    in_maps = []
    for c in range(N_CORES):
        in_maps.append({
            "x_sh": xf[c * TPC:(c + 1) * TPC],
            "wa_d": weight_A,
            "wb_d": weight_B,
            "bias_d": bias_row,
        })

    trace = os.environ.get("BASS_KERNEL_TRACE", "0") == "1"
    kwargs = {}
    if trace:
        _install_ntff_hook()
        kwargs["trace"] = True
        tmpdir = os.environ.get("BASS_KERNEL_TRACE_DIR")
        if tmpdir:
            os.makedirs(tmpdir, exist_ok=True)
            kwargs["tmpdir"] = tmpdir

    res = run_bass_kernel_spmd(nc, in_maps, core_ids=list(range(N_CORES)),
                               **kwargs)
    if trace:
        kernel.last_exec_time_ns = res.exec_time_ns

    out = np.empty((T_FULL, OUT_F), dtype=np.float32)
    for c in range(N_CORES):
        out[c * TPC:(c + 1) * TPC] = res.results[c]["out_d"]
    return out.reshape(*lead, OUT_F)


def _install_ntff_hook():
    """Provide antenv.axon_hooks (missing in this image) so trace=True works."""
    import types
    if "antenv.axon_hooks" in sys.modules:
        return
    try:
        from trn_agent_boot.trn_boot import _ntff_profile_via_ctypes
        hook = _ntff_profile_via_ctypes("/opt/axon/libaxon_pjrt.so")
    except Exception:
        hook = None
    mod = types.ModuleType("antenv.axon_hooks")
    mod.get_axon_ntff_profile_hook = lambda: hook
    mod.set_axon_ntff_profile_hook = lambda h: None
    import antenv  # noqa: F401
    sys.modules["antenv.axon_hooks"] = mod
